# revision 20
# baseline (speedup 1.0000x reference)
"""CodaPrompt kernel for Trainium2 (Bass/Tile) on 8 NeuronCores.

Math (reference):
    a[e,b,k,:] = x[b,:] * As[e,k,:]
    q = a / max(||a||_2, eps)        (normalize over d)
    nK = Ks / max(||Ks||_2, eps)
    aq[e,b,k] = <q[e,b,k,:], nK[e,k,:]>
    P_[e,b,l,:] = sum_k aq[e,b,k] * Ps[e,k,l,:]
    out = stack([P_[:,:, :L/2], P_[:,:, L/2:]])   # [2, E, B, L/2, D]

Sharding: SSPLIT L-slices x (8/SSPLIT) batch-slices (default 2x4). The cost
model charges DMA at an aggregate 360 GB/s per core, so the kernel is a pure
bytes-moved problem: the f32 formulation moved 40.75 MB/core (113 us floor).
All I/O is carried in fp16 instead (inputs cast on host, output stored fp16
and upcast on host) which halves traffic to ~21 MB/core (~59 us floor) at
~1e-3 relative error vs the 2e-2 gate. SSPLIT=2 (vs 4) costs +2.1 us of DMA
floor but leaves every compute engine 15 us+ of slack (PE 38 us, DVE ~44 us,
ACT ~34 us vs the 58.7 us DMA floor), so the DMA roofline is actually
reachable; at SSPLIT=4 the PE/DVE load (51/52 us) crowds the 56.6 us floor.

Device-side formulation (per core: batch slice of BC rows, one L-slice):
    num[e,k,b] = sum_d (As*nK)[e,k,d] * x[b,d]        -> matmul, contraction over d
    den2[e,k,b] = sum_d (As*As)[e,k,d] * x2[b,d]      -> matmul (x2 on device)
    aq[e,k,b] = num * rsqrt(den2)                      (ACT sqrt + DVE recip + mul)
    P_half[b, (l d)] = aq[e,:,b].T @ Ps[e, :, half]    -> matmul, contraction over k

PSUM P_ chunks are drained in [128,1024] pairs (halves the copy count and its
fixed access-latency overhead), alternating DVE/ACT so neither engine becomes
the bottleneck; one fp16 store per (e, m-chunk) keeps HWDGE overhead low.

Host prep is O(E*K*D) pool preprocessing (normalize Ks, fuse/transpose
weights, slice Ps halves, fp16 casts) plus the x transpose; all O(B*...)
FLOPs on device.
"""

import os
import sys
from contextlib import ExitStack

import numpy as np

if "/opt/trn_rl_repo" not in sys.path:
    sys.path.insert(0, "/opt/trn_rl_repo")

import concourse.mybir as mybir
from concourse import bacc, tile
from concourse.bass_utils import run_bass_kernel_spmd

B, D, E, K, L = 2048, 768, 5, 100, 8
NCORES = 8
SSPLIT = int(os.environ.get("CODA_SSPLIT", "2"))  # L-axis splits (2 or 4)
QSPLIT = NCORES // SSPLIT # batch splits
BC = B // QSPLIT          # batch rows per core
LH = L // SSPLIT          # l entries per core
DC = D // 128             # 6 contraction chunks of 128
NDH = LH * D              # P_ cols per core
NCHUNK = 512              # psum bank width in f32
NJ = NDH // NCHUNK        # n-chunks per core
MC = BC // 128            # output-partition chunks
NB = max(1, BC // 512)    # moving-operand chunks for num/den
EPS = 1e-12

F32 = mybir.dt.float32
# fp16 end-to-end: 1 cycle/row on PE (same as fp32r at N>=256) and half the
# DMA bytes of f32. Error ~1e-3 relative vs the 2e-2 gate.
MM_DTYPE = os.environ.get("CODA_MM_DTYPE", "float16")
MM_DT = getattr(mybir.dt, MM_DTYPE)
PS_DTYPE = os.environ.get("CODA_PS_DTYPE", MM_DTYPE)
PS_DT = getattr(mybir.dt, PS_DTYPE)
# Output dtype in DRAM; upcast to f32 on host. fp16 halves the dominant
# store traffic (15.7 MB/core vs 31.5 in f32).
OUT_DTYPE = os.environ.get("CODA_OUT_DTYPE", "float16")
OUT_DT = getattr(mybir.dt, OUT_DTYPE)


def _build_bass(repeat=1):
    # Bacc (not plain Bass): its finalize() runs move_matmul_waits_to_ldweights
    # + generate_event_semaphores, without which multi-dependency matmuls hit
    # walrus "Too many sync wait commands".
    nc = bacc.Bacc(None)

    xT_d = nc.declare_dram_parameter("xT", [D, BC], MM_DT, isOutput=False)
    w_d = nc.declare_dram_parameter("w12T", [2, D, E, K], MM_DT, isOutput=False)
    ps_d = nc.declare_dram_parameter("ps", [E, K, NDH], PS_DT, isOutput=False)
    out_d = nc.declare_dram_parameter("out", [E, BC, LH, D], OUT_DT, isOutput=True)

    with ExitStack() as ctx:
        tc = ctx.enter_context(tile.TileContext(nc))
        const = ctx.enter_context(tc.tile_pool(name="const", bufs=1))
        psp = ctx.enter_context(tc.tile_pool(name="psp", bufs=E))
        smallp = ctx.enter_context(tc.tile_pool(name="smallp", bufs=2))
        resp = ctx.enter_context(tc.tile_pool(name="resp", bufs=4))
        # PSUM budget (8 banks of 2KB/partition): num+den (1 bank each at
        # BC=512) x 1 buf + P_ group-tiles (GLEN banks) x 2 bufs. Two group
        # bufs let the P_ matmuls run a full drain-group ahead of the psum
        # copies, and the two in-flight groups drain on different engines
        # (DVE/ACT) concurrently, so the PE never stalls on a psum slot.
        GLEN = 2
        pndp = ctx.enter_context(tc.tile_pool(name="pndp", bufs=1, space="PSUM"))
        ppp = ctx.enter_context(
            tc.tile_pool(name="ppp", bufs=(3 if BC <= 512 else 2), space="PSUM")
        )

        # Resident operands: x quarter (transposed) and the fused W1=As*nK /
        # W2=As^2 weight block, chunked to 128 partitions. Per-chunk loads so
        # the first num/den matmuls start as soon as their own d-chunk lands.
        # x^2 is computed on-device (saves its DMA).
        # Single fused DMA per operand (768 descriptors each): per-chunk
        # loads serialize instructions through the single-slot HWDGE (625ns
        # each), which exceeds the actual transfer time of this phase.
        # Order x -> W1 -> W2 so the num matmuls (needing x+W1 only) start
        # ~2us earlier; the aq chain is the critical path to the first
        # output store, which must be ready before the pool loads drain.
        xT_r = xT_d[:].rearrange("(c p) b -> p c b", p=128)
        w1_r = w_d[0].rearrange("(c p) e k -> p c e k", p=128)
        w2_r = w_d[1].rearrange("(c p) e k -> p c e k", p=128)
        xs = const.tile([128, DC, BC], MM_DT, name="xs", tag="xs")
        x2s = const.tile([128, DC, BC], MM_DT, name="x2s", tag="x2s")
        ws1 = const.tile([128, DC, E, K], MM_DT, name="ws1", tag="ws1")
        ws2 = const.tile([128, DC, E, K], MM_DT, name="ws2", tag="ws2")
        # W2 before W1: den is emitted (and executed, PE is in-order) before
        # num, so its operands must land first. The sqrt+reciprocal chain on
        # den then overlaps the num matmuls instead of serializing after them.
        nc.sync.dma_start(xs[:], xT_r)
        nc.sync.dma_start(ws2[:], w2_r)
        nc.sync.dma_start(ws1[:], w1_r)
        # One x^2 op, not per-chunk: the PE p-state ramp resets on every
        # sequencer stall, so each den matmul waiting on its own x2 chunk
        # would repeatedly knock the PE clock back to 0.65 GHz.
        nc.vector.tensor_mul(
            x2s[:].rearrange("p c b -> p (c b)"),
            xs[:].rearrange("p c b -> p (c b)"),
            xs[:].rearrange("p c b -> p (c b)"),
        )

        for _ in range(repeat):
            # All pool loads issue upfront (own slots, bufs=E) so no load
            # ever queues behind output stores in a DMA FIFO.
            psts = []
            for e in range(E):
                pst = psp.tile([K, NDH], PS_DT, name="pst", tag="ps")
                nc.sync.dma_start(pst[:], ps_d[e])
                psts.append(pst)
            # The work is emitted as a flat stream of ~1.28us-of-PE units:
            # U(e,half) = den+sqrt+recip+num+aq for a 256-column batch half,
            # P(e,m) = one output m-chunk (6 matmuls + psum drain + store).
            # Interleaving U units between P units keeps PE work uniform —
            # emitting an e's num/den as one block creates a ~2.6us
            # compute-only burst at each e boundary during which no psum
            # drains complete and the store stream (the DMA bottleneck)
            # starves. The halved batch also shortens the critical path to
            # the very first store, which must beat the input-load drain.
            ncopy = 0
            BH = 256
            NBH = BC // BH
            MH = MC // NBH  # m-chunks fed per half
            tiles = {}

            def emit_u(e, nb):
                if nb == 0:
                    tiles[e] = (
                        pndp.tile([K, BC], F32, name="num", tag="num"),
                        pndp.tile([K, BC], F32, name="den", tag="den"),
                        smallp.tile([K, BC], F32, name="sden", tag="sden"),
                        smallp.tile([K, BC], F32, name="rden", tag="rden"),
                        smallp.tile([K, BC], PS_DT, name="aq", tag="aq", bufs=2),
                    )
                num, den, sden, rden, aq = tiles[e]
                bsl = slice(nb * BH, (nb + 1) * BH)
                # den first: its sqrt+reciprocal tail overlaps the num
                # matmuls that follow on the in-order PE.
                for c in range(DC):
                    nc.tensor.matmul(
                        den[:, bsl],
                        ws2[:, c, e, :],
                        x2s[:, c, bsl],
                        start=(c == 0),
                        stop=(c == DC - 1),
                    )
                # aq = num / sqrt(den2) per b-chunk (den2 >> eps^2 here).
                nc.scalar.sqrt(sden[:, bsl], den[:, bsl])
                nc.vector.reciprocal(rden[:, bsl], sden[:, bsl])
                for c in range(DC):
                    nc.tensor.matmul(
                        num[:, bsl],
                        ws1[:, c, e, :],
                        xs[:, c, bsl],
                        start=(c == 0),
                        stop=(c == DC - 1),
                    )
                nc.vector.tensor_mul(aq[:, bsl], num[:, bsl], rden[:, bsl])

            def emit_p(e, m):
                nonlocal ncopy
                aq = tiles[e][4]
                pst = psts[e]
                res = resp.tile([128, NDH], OUT_DT, name="res", tag="res")
                j = 0
                while j < NJ:
                    glen = min(GLEN, NJ - j)
                    pp = ppp.tile([128, GLEN * NCHUNK], F32, name="pp", tag="pp")
                    for h in range(glen):
                        nc.tensor.matmul(
                            pp[:, h * NCHUNK : (h + 1) * NCHUNK],
                            aq[:, m * 128 : (m + 1) * 128],
                            pst[:, (j + h) * NCHUNK : (j + h + 1) * NCHUNK],
                            start=True,
                            stop=True,
                        )
                    dst = res[:, j * NCHUNK : (j + glen) * NCHUNK]
                    src = pp[:, : glen * NCHUNK]
                    # Split the psum drain 4:5 between DVE and ACT: DVE also
                    # carries the reciprocal + aq multiply (and its copies
                    # cost 1192ns vs ACT's 1038), so a strict 1:1 alternation
                    # saturates DVE while ACT idles ~20%.
                    if (ncopy % 9) % 2 == 1:
                        nc.vector.tensor_copy(dst, src)
                    else:
                        nc.scalar.copy(dst, src)
                    ncopy += 1
                    j += glen
                out_ap = out_d[e, m * 128 : (m + 1) * 128, :, :].rearrange(
                    "b l d -> b (l d)"
                )
                nc.sync.dma_start(out_ap[:], res[:])

            # Pipelined unit order (NBH=2, MH=2): U(e,0) runs two P-units
            # ahead of its consumers; P(e-1,3) overlaps the next e's first
            # half-unit. PSUM slot reuse (pndp bufs=1) is satisfied: U(e,0)
            # writes den/num only after U(e-1,1)'s sqrt/aq-mul have read the
            # previous e's tiles, three units earlier.
            assert NBH == 2 and MH == 2, (NBH, MH)
            for e in range(E):
                emit_u(e, 0)
                if e > 0:
                    emit_p(e - 1, 3)
                emit_p(e, 0)
                emit_u(e, 1)
                emit_p(e, 1)
                emit_p(e, 2)
            emit_p(E - 1, 3)

    if not nc.is_finalized():
        nc.finalize()
    return nc


_NC_CACHE = None


def _get_nc():
    global _NC_CACHE
    if _NC_CACHE is None:
        _NC_CACHE = _build_bass()
    return _NC_CACHE


def _prep_inputs(x, Ks, As, Ps):
    x = np.asarray(x, dtype=np.float32)
    Ks = np.asarray(Ks, dtype=np.float32)
    As = np.asarray(As, dtype=np.float32)
    Ps = np.asarray(Ps, dtype=np.float32)

    mm_np = mybir.dt.np(MM_DT)
    ps_np = mybir.dt.np(PS_DT)

    nrm = np.sqrt(np.sum(Ks * Ks, axis=-1, keepdims=True))
    nK = Ks / np.maximum(nrm, EPS)
    w12T = np.empty((2, D, E, K), dtype=np.float32)
    w12T[0] = (As * nK).transpose(2, 0, 1)
    w12T[1] = (As * As).transpose(2, 0, 1)
    w12T = w12T.astype(mm_np, copy=False)

    ps_slices = [
        np.ascontiguousarray(
            Ps[:, :, si * LH : (si + 1) * LH, :].reshape(E, K, NDH)
        ).astype(ps_np, copy=False)
        for si in range(SSPLIT)
    ]
    xT = np.ascontiguousarray(x.T).astype(mm_np, copy=False)  # [D, B]

    in_maps = []
    for c in range(NCORES):
        si, q = divmod(c, QSPLIT)
        in_maps.append(
            {
                "xT": np.ascontiguousarray(xT[:, q * BC : (q + 1) * BC]),
                "w12T": w12T,
                "ps": ps_slices[si],
            }
        )
    return in_maps


def _run(x, Ks, As, Ps, trace=False, **spmd_kwargs):
    nc = _get_nc()
    in_maps = _prep_inputs(x, Ks, As, Ps)
    res = run_bass_kernel_spmd(nc, in_maps, list(range(NCORES)), trace=trace, **spmd_kwargs)
    out = np.empty((2, E, B, L // 2, D), dtype=np.float32)
    for c in range(NCORES):
        si, q = divmod(c, QSPLIT)
        s, lp = divmod(si * LH, L // 2)
        out[s, :, q * BC : (q + 1) * BC, lp : lp + LH] = res.results[c]["out"].astype(
            np.float32
        )
    return out, res


def kernel(x, Ks, As, Ps):
    out, _ = _run(x, Ks, As, Ps, trace=False)
    return out


# revision 26
# speedup vs baseline: 1.0333x; 1.0333x over previous
"""CodaPrompt kernel for Trainium2 (Bass/Tile) on 8 NeuronCores.

Math (reference):
    a[e,b,k,:] = x[b,:] * As[e,k,:]
    q = a / max(||a||_2, eps)        (normalize over d)
    nK = Ks / max(||Ks||_2, eps)
    aq[e,b,k] = <q[e,b,k,:], nK[e,k,:]>
    P_[e,b,l,:] = sum_k aq[e,b,k] * Ps[e,k,l,:]
    out = stack([P_[:,:, :L/2], P_[:,:, L/2:]])   # [2, E, B, L/2, D]

Sharding: SSPLIT L-slices x (8/SSPLIT) batch-slices (default 2x4). The cost
model charges DMA at an aggregate 360 GB/s per core, so the kernel is a pure
bytes-moved problem: the f32 formulation moved 40.75 MB/core (113 us floor).
All I/O is carried in fp16 instead (inputs cast on host, output stored fp16
and upcast on host) which halves traffic to ~21 MB/core (~59 us floor) at
~1e-3 relative error vs the 2e-2 gate. SSPLIT=2 (vs 4) costs +2.1 us of DMA
floor but leaves every compute engine 15 us+ of slack (PE 38 us, DVE ~44 us,
ACT ~34 us vs the 58.7 us DMA floor), so the DMA roofline is actually
reachable; at SSPLIT=4 the PE/DVE load (51/52 us) crowds the 56.6 us floor.

Device-side formulation (per core: batch slice of BC rows, one L-slice):
    num[e,k,b] = sum_d (As*nK)[e,k,d] * x[b,d]        -> matmul, contraction over d
    den2[e,k,b] = sum_d (As*As)[e,k,d] * x2[b,d]      -> matmul (x2 on device)
    aq[e,k,b] = num * rsqrt(den2)                      (ACT sqrt + DVE recip + mul)
    P_half[b, (l d)] = aq[e,:,b].T @ Ps[e, :, half]    -> matmul, contraction over k

PSUM P_ chunks are drained in [128,1024] pairs (halves the copy count and its
fixed access-latency overhead), alternating DVE/ACT so neither engine becomes
the bottleneck; one fp16 store per (e, m-chunk) keeps HWDGE overhead low.

Host prep is O(E*K*D) pool preprocessing (normalize Ks, fuse/transpose
weights, slice Ps halves, fp16 casts) plus the x transpose; all O(B*...)
FLOPs on device.
"""

import os
import sys
from contextlib import ExitStack

import numpy as np

if "/opt/trn_rl_repo" not in sys.path:
    sys.path.insert(0, "/opt/trn_rl_repo")

import concourse.mybir as mybir
from concourse import bacc, tile
from concourse.bass_utils import run_bass_kernel_spmd

B, D, E, K, L = 2048, 768, 5, 100, 8
NCORES = 8
SSPLIT = int(os.environ.get("CODA_SSPLIT", "2"))  # L-axis splits (2 or 4)
QSPLIT = NCORES // SSPLIT # batch splits
BC = B // QSPLIT          # batch rows per core
LH = L // SSPLIT          # l entries per core
DC = D // 128             # 6 contraction chunks of 128
NDH = LH * D              # P_ cols per core
NCHUNK = 512              # psum bank width in f32
NJ = NDH // NCHUNK        # n-chunks per core
MC = BC // 128            # output-partition chunks
NB = max(1, BC // 512)    # moving-operand chunks for num/den
EPS = 1e-12

F32 = mybir.dt.float32
# fp16 end-to-end: 1 cycle/row on PE (same as fp32r at N>=256) and half the
# DMA bytes of f32. Error ~1e-3 relative vs the 2e-2 gate.
MM_DTYPE = os.environ.get("CODA_MM_DTYPE", "float16")
MM_DT = getattr(mybir.dt, MM_DTYPE)
PS_DTYPE = os.environ.get("CODA_PS_DTYPE", MM_DTYPE)
PS_DT = getattr(mybir.dt, PS_DTYPE)
# Output dtype in DRAM; upcast to f32 on host. fp16 halves the dominant
# store traffic (15.7 MB/core vs 31.5 in f32).
OUT_DTYPE = os.environ.get("CODA_OUT_DTYPE", "float16")
OUT_DT = getattr(mybir.dt, OUT_DTYPE)
# The den weights (As^2) and x^2 carry only the rsqrt normalization scale;
# fp8e4m3's ~4%/element noise averages to ~0.3% over the 768-term positive
# sum (~0.15% on aq), far inside the error budget, and halves the W2 load.
W2_DTYPE = os.environ.get("CODA_W2_DTYPE", "float8e4")
W2_DT = getattr(mybir.dt, W2_DTYPE)
# K padded so fp8 DMA rows stay >= 512B (below that the cost model charges
# 2x per descriptor for read-modify-write).
KP = 104 if mybir.dt.size(W2_DT) == 1 else K


def _build_bass(repeat=1):
    # Bacc (not plain Bass): its finalize() runs move_matmul_waits_to_ldweights
    # + generate_event_semaphores, without which multi-dependency matmuls hit
    # walrus "Too many sync wait commands".
    nc = bacc.Bacc(None)

    xT_d = nc.declare_dram_parameter("xT", [D, BC], MM_DT, isOutput=False)
    w1_d = nc.declare_dram_parameter("w1T", [D, E, K], MM_DT, isOutput=False)
    w2_d = nc.declare_dram_parameter("w2T", [D, E, KP], W2_DT, isOutput=False)
    ps_d = nc.declare_dram_parameter("ps", [E, K, NDH], PS_DT, isOutput=False)
    out_d = nc.declare_dram_parameter("out", [E, BC, LH, D], OUT_DT, isOutput=True)

    with ExitStack() as ctx:
        tc = ctx.enter_context(tile.TileContext(nc))
        const = ctx.enter_context(tc.tile_pool(name="const", bufs=1))
        psp = ctx.enter_context(tc.tile_pool(name="psp", bufs=E))
        smallp = ctx.enter_context(tc.tile_pool(name="smallp", bufs=2))
        resp = ctx.enter_context(tc.tile_pool(name="resp", bufs=4))
        # PSUM budget (8 banks of 2KB/partition): num+den (1 bank each at
        # BC=512) x 1 buf + P_ group-tiles (GLEN banks) x 2 bufs. Two group
        # bufs let the P_ matmuls run a full drain-group ahead of the psum
        # copies, and the two in-flight groups drain on different engines
        # (DVE/ACT) concurrently, so the PE never stalls on a psum slot.
        GLEN = 2
        pndp = ctx.enter_context(tc.tile_pool(name="pndp", bufs=1, space="PSUM"))
        ppp = ctx.enter_context(
            tc.tile_pool(name="ppp", bufs=(3 if BC <= 512 else 2), space="PSUM")
        )

        # Resident operands: x quarter (transposed) and the fused W1=As*nK /
        # W2=As^2 weight block, chunked to 128 partitions. Per-chunk loads so
        # the first num/den matmuls start as soon as their own d-chunk lands.
        # x^2 is computed on-device (saves its DMA).
        # Single fused DMA per operand (768 descriptors each): per-chunk
        # loads serialize instructions through the single-slot HWDGE (625ns
        # each), which exceeds the actual transfer time of this phase.
        # Order x -> W1 -> W2 so the num matmuls (needing x+W1 only) start
        # ~2us earlier; the aq chain is the critical path to the first
        # output store, which must be ready before the pool loads drain.
        xT_r = xT_d[:].rearrange("(c p) b -> p c b", p=128)
        w1_r = w1_d[:].rearrange("(c p) e k -> p c e k", p=128)
        w2_r = w2_d[:].rearrange("(c p) e k -> p c e k", p=128)
        xs = const.tile([128, DC, BC], MM_DT, name="xs", tag="xs")
        x2s = const.tile([128, DC, BC], W2_DT, name="x2s", tag="x2s")
        ws1 = const.tile([128, DC, E, K], MM_DT, name="ws1", tag="ws1")
        ws2 = const.tile([128, DC, E, KP], W2_DT, name="ws2", tag="ws2")
        # W2 before W1: den is emitted (and executed, PE is in-order) before
        # num, so its operands must land first. The sqrt+reciprocal chain on
        # den then overlaps the num matmuls instead of serializing after them.
        nc.sync.dma_start(xs[:], xT_r)
        nc.sync.dma_start(ws2[:], w2_r)
        nc.sync.dma_start(ws1[:], w1_r)
        # One x^2 op, not per-chunk: the PE p-state ramp resets on every
        # sequencer stall, so each den matmul waiting on its own x2 chunk
        # would repeatedly knock the PE clock back to 0.65 GHz.
        nc.vector.tensor_mul(
            x2s[:].rearrange("p c b -> p (c b)"),
            xs[:].rearrange("p c b -> p (c b)"),
            xs[:].rearrange("p c b -> p (c b)"),
        )

        for _ in range(repeat):
            # All pool loads issue upfront (own slots, bufs=E) so no load
            # ever queues behind output stores in a DMA FIFO.
            psts = []
            for e in range(E):
                pst = psp.tile([K, NDH], PS_DT, name="pst", tag="ps")
                nc.sync.dma_start(pst[:], ps_d[e])
                psts.append(pst)
            # The work is emitted as a flat stream of ~1.28us-of-PE units:
            # U(e,half) = den+sqrt+recip+num+aq for a 256-column batch half,
            # P(e,m) = one output m-chunk (6 matmuls + psum drain + store).
            # Interleaving U units between P units keeps PE work uniform —
            # emitting an e's num/den as one block creates a ~2.6us
            # compute-only burst at each e boundary during which no psum
            # drains complete and the store stream (the DMA bottleneck)
            # starves. The halved batch also shortens the critical path to
            # the very first store, which must beat the input-load drain.
            ncopy = 0
            BH = 256
            NBH = BC // BH
            MH = MC // NBH  # m-chunks fed per half
            tiles = {}

            def emit_u(e, nb):
                if nb == 0:
                    tiles[e] = (
                        pndp.tile([K, BC], F32, name="num", tag="num"),
                        pndp.tile([K, BC], F32, name="den", tag="den"),
                        smallp.tile([K, BC], F32, name="sden", tag="sden"),
                        smallp.tile([K, BC], F32, name="rden", tag="rden"),
                        smallp.tile([K, BC], PS_DT, name="aq", tag="aq", bufs=2),
                    )
                num, den, sden, rden, aq = tiles[e]
                bsl = slice(nb * BH, (nb + 1) * BH)
                # den first: its sqrt+reciprocal tail overlaps the num
                # matmuls that follow on the in-order PE.
                for c in range(DC):
                    nc.tensor.matmul(
                        den[:, bsl],
                        ws2[:, c, e, :K],
                        x2s[:, c, bsl],
                        start=(c == 0),
                        stop=(c == DC - 1),
                    )
                # aq = num / sqrt(den2) per b-chunk (den2 >> eps^2 here).
                nc.scalar.sqrt(sden[:, bsl], den[:, bsl])
                nc.vector.reciprocal(rden[:, bsl], sden[:, bsl])
                for c in range(DC):
                    nc.tensor.matmul(
                        num[:, bsl],
                        ws1[:, c, e, :],
                        xs[:, c, bsl],
                        start=(c == 0),
                        stop=(c == DC - 1),
                    )
                nc.vector.tensor_mul(aq[:, bsl], num[:, bsl], rden[:, bsl])

            def emit_p(e, m):
                nonlocal ncopy
                aq = tiles[e][4]
                pst = psts[e]
                res = resp.tile([128, NDH], OUT_DT, name="res", tag="res")
                j = 0
                while j < NJ:
                    glen = min(GLEN, NJ - j)
                    pp = ppp.tile([128, GLEN * NCHUNK], F32, name="pp", tag="pp")
                    for h in range(glen):
                        nc.tensor.matmul(
                            pp[:, h * NCHUNK : (h + 1) * NCHUNK],
                            aq[:, m * 128 : (m + 1) * 128],
                            pst[:, (j + h) * NCHUNK : (j + h + 1) * NCHUNK],
                            start=True,
                            stop=True,
                        )
                    dst = res[:, j * NCHUNK : (j + glen) * NCHUNK]
                    src = pp[:, : glen * NCHUNK]
                    # Split the psum drain 4:5 between DVE and ACT: DVE also
                    # carries the reciprocal + aq multiply (and its copies
                    # cost 1192ns vs ACT's 1038), so a strict 1:1 alternation
                    # saturates DVE while ACT idles ~20%.
                    if (ncopy % 9) % 2 == 1:
                        nc.vector.tensor_copy(dst, src)
                    else:
                        nc.scalar.copy(dst, src)
                    ncopy += 1
                    j += glen
                out_ap = out_d[e, m * 128 : (m + 1) * 128, :, :].rearrange(
                    "b l d -> b (l d)"
                )
                nc.sync.dma_start(out_ap[:], res[:])

            # Pipelined unit order (NBH=2, MH=2): U(e,0) runs two P-units
            # ahead of its consumers; P(e-1,3) overlaps the next e's first
            # half-unit. PSUM slot reuse (pndp bufs=1) is satisfied: U(e,0)
            # writes den/num only after U(e-1,1)'s sqrt/aq-mul have read the
            # previous e's tiles, three units earlier.
            assert NBH == 2 and MH == 2, (NBH, MH)
            for e in range(E):
                emit_u(e, 0)
                if e > 0:
                    emit_p(e - 1, 3)
                emit_p(e, 0)
                emit_u(e, 1)
                emit_p(e, 1)
                emit_p(e, 2)
            emit_p(E - 1, 3)

    if not nc.is_finalized():
        nc.finalize()
    return nc


_NC_CACHE = None


def _get_nc():
    global _NC_CACHE
    if _NC_CACHE is None:
        _NC_CACHE = _build_bass()
    return _NC_CACHE


def _prep_inputs(x, Ks, As, Ps):
    x = np.asarray(x, dtype=np.float32)
    Ks = np.asarray(Ks, dtype=np.float32)
    As = np.asarray(As, dtype=np.float32)
    Ps = np.asarray(Ps, dtype=np.float32)

    mm_np = mybir.dt.np(MM_DT)
    ps_np = mybir.dt.np(PS_DT)

    nrm = np.sqrt(np.sum(Ks * Ks, axis=-1, keepdims=True))
    nK = Ks / np.maximum(nrm, EPS)
    w1T = np.ascontiguousarray((As * nK).transpose(2, 0, 1)).astype(mm_np, copy=False)
    w2T = np.zeros((D, E, KP), dtype=mybir.dt.np(W2_DT))
    w2T[:, :, :K] = (As * As).transpose(2, 0, 1).astype(w2T.dtype)

    ps_slices = [
        np.ascontiguousarray(
            Ps[:, :, si * LH : (si + 1) * LH, :].reshape(E, K, NDH)
        ).astype(ps_np, copy=False)
        for si in range(SSPLIT)
    ]
    xT = np.ascontiguousarray(x.T).astype(mm_np, copy=False)  # [D, B]

    in_maps = []
    for c in range(NCORES):
        si, q = divmod(c, QSPLIT)
        in_maps.append(
            {
                "xT": np.ascontiguousarray(xT[:, q * BC : (q + 1) * BC]),
                "w1T": w1T,
                "w2T": w2T,
                "ps": ps_slices[si],
            }
        )
    return in_maps


def _run(x, Ks, As, Ps, trace=False, **spmd_kwargs):
    nc = _get_nc()
    in_maps = _prep_inputs(x, Ks, As, Ps)
    res = run_bass_kernel_spmd(nc, in_maps, list(range(NCORES)), trace=trace, **spmd_kwargs)
    out = np.empty((2, E, B, L // 2, D), dtype=np.float32)
    for c in range(NCORES):
        si, q = divmod(c, QSPLIT)
        s, lp = divmod(si * LH, L // 2)
        out[s, :, q * BC : (q + 1) * BC, lp : lp + LH] = res.results[c]["out"].astype(
            np.float32
        )
    return out, res


def kernel(x, Ks, As, Ps):
    out, _ = _run(x, Ks, As, Ps, trace=False)
    return out


# revision 28
# speedup vs baseline: 1.0544x; 1.0203x over previous
"""CodaPrompt kernel for Trainium2 (Bass/Tile) on 8 NeuronCores.

Math (reference):
    a[e,b,k,:] = x[b,:] * As[e,k,:]
    q = a / max(||a||_2, eps)        (normalize over d)
    nK = Ks / max(||Ks||_2, eps)
    aq[e,b,k] = <q[e,b,k,:], nK[e,k,:]>
    P_[e,b,l,:] = sum_k aq[e,b,k] * Ps[e,k,l,:]
    out = stack([P_[:,:, :L/2], P_[:,:, L/2:]])   # [2, E, B, L/2, D]

Sharding: SSPLIT L-slices x (8/SSPLIT) batch-slices (default 2x4). The cost
model charges DMA at an aggregate 360 GB/s per core, so the kernel is a pure
bytes-moved problem: the f32 formulation moved 40.75 MB/core (113 us floor).
All I/O is carried in fp16 instead (inputs cast on host, output stored fp16
and upcast on host) which halves traffic to ~21 MB/core (~59 us floor) at
~1e-3 relative error vs the 2e-2 gate. SSPLIT=2 (vs 4) costs +2.1 us of DMA
floor but leaves every compute engine 15 us+ of slack (PE 38 us, DVE ~44 us,
ACT ~34 us vs the 58.7 us DMA floor), so the DMA roofline is actually
reachable; at SSPLIT=4 the PE/DVE load (51/52 us) crowds the 56.6 us floor.

Device-side formulation (per core: batch slice of BC rows, one L-slice):
    num[e,k,b] = sum_d (As*nK)[e,k,d] * x[b,d]        -> matmul, contraction over d
    den2[e,k,b] = sum_d (As*As)[e,k,d] * x2[b,d]      -> matmul (x2 on device)
    aq[e,k,b] = num * rsqrt(den2)                      (ACT sqrt + DVE recip + mul)
    P_half[b, (l d)] = aq[e,:,b].T @ Ps[e, :, half]    -> matmul, contraction over k

PSUM P_ chunks are drained in [128,1024] pairs (halves the copy count and its
fixed access-latency overhead), alternating DVE/ACT so neither engine becomes
the bottleneck; one fp16 store per (e, m-chunk) keeps HWDGE overhead low.

Host prep is O(E*K*D) pool preprocessing (normalize Ks, fuse/transpose
weights, slice Ps halves, fp16 casts) plus the x transpose; all O(B*...)
FLOPs on device.
"""

import os
import sys
from contextlib import ExitStack

import numpy as np

if "/opt/trn_rl_repo" not in sys.path:
    sys.path.insert(0, "/opt/trn_rl_repo")

import concourse.mybir as mybir
from concourse import bacc, tile
from concourse.bass_utils import run_bass_kernel_spmd

B, D, E, K, L = 2048, 768, 5, 100, 8
NCORES = 8
SSPLIT = int(os.environ.get("CODA_SSPLIT", "2"))  # L-axis splits (2 or 4)
QSPLIT = NCORES // SSPLIT # batch splits
BC = B // QSPLIT          # batch rows per core
LH = L // SSPLIT          # l entries per core
DC = D // 128             # 6 contraction chunks of 128
NDH = LH * D              # P_ cols per core
NCHUNK = 512              # psum bank width in f32
NJ = NDH // NCHUNK        # n-chunks per core
MC = BC // 128            # output-partition chunks
NB = max(1, BC // 512)    # moving-operand chunks for num/den
EPS = 1e-12

F32 = mybir.dt.float32
# fp16 end-to-end: 1 cycle/row on PE (same as fp32r at N>=256) and half the
# DMA bytes of f32. Error ~1e-3 relative vs the 2e-2 gate.
MM_DTYPE = os.environ.get("CODA_MM_DTYPE", "float16")
MM_DT = getattr(mybir.dt, MM_DTYPE)
PS_DTYPE = os.environ.get("CODA_PS_DTYPE", MM_DTYPE)
PS_DT = getattr(mybir.dt, PS_DTYPE)
# Output dtype in DRAM; upcast to f32 on host. fp16 halves the dominant
# store traffic (15.7 MB/core vs 31.5 in f32).
OUT_DTYPE = os.environ.get("CODA_OUT_DTYPE", "float16")
OUT_DT = getattr(mybir.dt, OUT_DTYPE)
# The den weights (As^2) and x^2 carry only the rsqrt normalization scale;
# fp8e4m3's ~4%/element noise averages to ~0.3% over the 768-term positive
# sum (~0.15% on aq), far inside the error budget, and halves the W2 load.
W2_DTYPE = os.environ.get("CODA_W2_DTYPE", "float8e4")
W2_DT = getattr(mybir.dt, W2_DTYPE)
# K padded so fp8 DMA rows stay >= 512B (below that the cost model charges
# 2x per descriptor for read-modify-write).
KP = 104 if mybir.dt.size(W2_DT) == 1 else K


def _build_bass(repeat=1):
    # Bacc (not plain Bass): its finalize() runs move_matmul_waits_to_ldweights
    # + generate_event_semaphores, without which multi-dependency matmuls hit
    # walrus "Too many sync wait commands".
    nc = bacc.Bacc(None)

    xT_d = nc.declare_dram_parameter("xT", [D, BC], MM_DT, isOutput=False)
    w1_d = nc.declare_dram_parameter("w1T", [D, E, K], MM_DT, isOutput=False)
    w2_d = nc.declare_dram_parameter("w2T", [D, E, KP], W2_DT, isOutput=False)
    ps_d = nc.declare_dram_parameter("ps", [E, K, NDH], PS_DT, isOutput=False)
    out_d = nc.declare_dram_parameter("out", [E, BC, LH, D], OUT_DT, isOutput=True)

    with ExitStack() as ctx:
        tc = ctx.enter_context(tile.TileContext(nc))
        const = ctx.enter_context(tc.tile_pool(name="const", bufs=1))
        psp = ctx.enter_context(tc.tile_pool(name="psp", bufs=E))
        smallp = ctx.enter_context(tc.tile_pool(name="smallp", bufs=2))
        resp = ctx.enter_context(tc.tile_pool(name="resp", bufs=4))
        # PSUM budget (8 banks of 2KB/partition): num+den (1 bank each at
        # BC=512) x 1 buf + P_ group-tiles (GLEN banks) x 2 bufs. Two group
        # bufs let the P_ matmuls run a full drain-group ahead of the psum
        # copies, and the two in-flight groups drain on different engines
        # (DVE/ACT) concurrently, so the PE never stalls on a psum slot.
        GLEN = 2
        pndp = ctx.enter_context(tc.tile_pool(name="pndp", bufs=1, space="PSUM"))
        ppp = ctx.enter_context(
            tc.tile_pool(name="ppp", bufs=(3 if BC <= 512 else 2), space="PSUM")
        )

        # Optional PE p-state warmup: dummy accumulating matmuls emitted
        # before the loads keep the PE busy until the first real matmul's
        # operands land; matmuls issued right after a long-running chain are
        # costed at the full 2.4 GHz clock instead of the 1.2 GHz ramp state.
        NWARM = int(os.environ.get("CODA_NWARM", "4"))
        if NWARM:
            warm = const.tile([128, 512], MM_DT, name="warm", tag="warm")
            nc.vector.memset(warm[:], 1.0)
            pwarm = ppp.tile([128, GLEN * NCHUNK], F32, name="pp", tag="pp")
            for i in range(NWARM):
                nc.tensor.matmul(
                    pwarm[:, :NCHUNK],
                    warm[:, :128],
                    warm[:],
                    start=(i == 0),
                    stop=(i == NWARM - 1),
                )

        # Resident operands: x quarter (transposed) and the fused W1=As*nK /
        # W2=As^2 weight block, chunked to 128 partitions. Per-chunk loads so
        # the first num/den matmuls start as soon as their own d-chunk lands.
        # x^2 is computed on-device (saves its DMA).
        # Single fused DMA per operand (768 descriptors each): per-chunk
        # loads serialize instructions through the single-slot HWDGE (625ns
        # each), which exceeds the actual transfer time of this phase.
        # Order x -> W1 -> W2 so the num matmuls (needing x+W1 only) start
        # ~2us earlier; the aq chain is the critical path to the first
        # output store, which must be ready before the pool loads drain.
        xT_r = xT_d[:].rearrange("(c p) b -> p c b", p=128)
        w1_r = w1_d[:].rearrange("(c p) e k -> p c e k", p=128)
        w2_r = w2_d[:].rearrange("(c p) e k -> p c e k", p=128)
        xs = const.tile([128, DC, BC], MM_DT, name="xs", tag="xs")
        x2s = const.tile([128, DC, BC], W2_DT, name="x2s", tag="x2s")
        ws1 = const.tile([128, DC, E, K], MM_DT, name="ws1", tag="ws1")
        ws2 = const.tile([128, DC, E, KP], W2_DT, name="ws2", tag="ws2")
        # W2 before W1: den is emitted (and executed, PE is in-order) before
        # num, so its operands must land first. The sqrt+reciprocal chain on
        # den then overlaps the num matmuls instead of serializing after them.
        nc.sync.dma_start(xs[:], xT_r)
        nc.sync.dma_start(ws2[:], w2_r)
        nc.sync.dma_start(ws1[:], w1_r)
        # One x^2 op, not per-chunk: the PE p-state ramp resets on every
        # sequencer stall, so each den matmul waiting on its own x2 chunk
        # would repeatedly knock the PE clock back to 0.65 GHz.
        nc.vector.tensor_mul(
            x2s[:].rearrange("p c b -> p (c b)"),
            xs[:].rearrange("p c b -> p (c b)"),
            xs[:].rearrange("p c b -> p (c b)"),
        )

        for _ in range(repeat):
            # All pool loads issue upfront (own slots, bufs=E) so no load
            # ever queues behind output stores in a DMA FIFO.
            psts = []
            for e in range(E):
                pst = psp.tile([K, NDH], PS_DT, name="pst", tag="ps")
                nc.sync.dma_start(pst[:], ps_d[e])
                psts.append(pst)
            # The work is emitted as a flat stream of ~1.28us-of-PE units:
            # U(e,half) = den+sqrt+recip+num+aq for a 256-column batch half,
            # P(e,m) = one output m-chunk (6 matmuls + psum drain + store).
            # Interleaving U units between P units keeps PE work uniform —
            # emitting an e's num/den as one block creates a ~2.6us
            # compute-only burst at each e boundary during which no psum
            # drains complete and the store stream (the DMA bottleneck)
            # starves. The halved batch also shortens the critical path to
            # the very first store, which must beat the input-load drain.
            ncopy = 0
            BH = 256
            NBH = BC // BH
            MH = MC // NBH  # m-chunks fed per half
            tiles = {}

            def emit_u(e, nb):
                if nb == 0:
                    tiles[e] = (
                        pndp.tile([K, BC], F32, name="num", tag="num"),
                        pndp.tile([K, BC], F32, name="den", tag="den"),
                        smallp.tile([K, BC], F32, name="sden", tag="sden"),
                        smallp.tile([K, BC], F32, name="rden", tag="rden"),
                        smallp.tile([K, BC], PS_DT, name="aq", tag="aq", bufs=2),
                    )
                num, den, sden, rden, aq = tiles[e]
                bsl = slice(nb * BH, (nb + 1) * BH)
                # den first: its sqrt+reciprocal tail overlaps the num
                # matmuls that follow on the in-order PE.
                for c in range(DC):
                    nc.tensor.matmul(
                        den[:, bsl],
                        ws2[:, c, e, :K],
                        x2s[:, c, bsl],
                        start=(c == 0),
                        stop=(c == DC - 1),
                    )
                # aq = num / sqrt(den2) per b-chunk (den2 >> eps^2 here).
                nc.scalar.sqrt(sden[:, bsl], den[:, bsl])
                nc.vector.reciprocal(rden[:, bsl], sden[:, bsl])
                for c in range(DC):
                    nc.tensor.matmul(
                        num[:, bsl],
                        ws1[:, c, e, :],
                        xs[:, c, bsl],
                        start=(c == 0),
                        stop=(c == DC - 1),
                    )
                nc.vector.tensor_mul(aq[:, bsl], num[:, bsl], rden[:, bsl])

            def emit_p(e, m):
                nonlocal ncopy
                aq = tiles[e][4]
                pst = psts[e]
                res = resp.tile([128, NDH], OUT_DT, name="res", tag="res")
                j = 0
                while j < NJ:
                    glen = min(GLEN, NJ - j)
                    pp = ppp.tile([128, GLEN * NCHUNK], F32, name="pp", tag="pp")
                    for h in range(glen):
                        nc.tensor.matmul(
                            pp[:, h * NCHUNK : (h + 1) * NCHUNK],
                            aq[:, m * 128 : (m + 1) * 128],
                            pst[:, (j + h) * NCHUNK : (j + h + 1) * NCHUNK],
                            start=True,
                            stop=True,
                        )
                    dst = res[:, j * NCHUNK : (j + glen) * NCHUNK]
                    src = pp[:, : glen * NCHUNK]
                    # Split the psum drain 4:5 between DVE and ACT: DVE also
                    # carries the reciprocal + aq multiply (and its copies
                    # cost 1192ns vs ACT's 1038), so a strict 1:1 alternation
                    # saturates DVE while ACT idles ~20%.
                    if (ncopy % 9) % 2 == 1:
                        nc.vector.tensor_copy(dst, src)
                    else:
                        nc.scalar.copy(dst, src)
                    ncopy += 1
                    j += glen
                out_ap = out_d[e, m * 128 : (m + 1) * 128, :, :].rearrange(
                    "b l d -> b (l d)"
                )
                nc.sync.dma_start(out_ap[:], res[:])

            # Pipelined unit order (NBH=2, MH=2): U(e,0) runs two P-units
            # ahead of its consumers; P(e-1,3) overlaps the next e's first
            # half-unit. PSUM slot reuse (pndp bufs=1) is satisfied: U(e,0)
            # writes den/num only after U(e-1,1)'s sqrt/aq-mul have read the
            # previous e's tiles, three units earlier.
            assert NBH == 2 and MH == 2, (NBH, MH)
            for e in range(E):
                emit_u(e, 0)
                if e > 0:
                    emit_p(e - 1, 3)
                emit_p(e, 0)
                emit_u(e, 1)
                emit_p(e, 1)
                emit_p(e, 2)
            emit_p(E - 1, 3)

    if not nc.is_finalized():
        nc.finalize()
    return nc


_NC_CACHE = None


def _get_nc():
    global _NC_CACHE
    if _NC_CACHE is None:
        _NC_CACHE = _build_bass()
    return _NC_CACHE


def _prep_inputs(x, Ks, As, Ps):
    x = np.asarray(x, dtype=np.float32)
    Ks = np.asarray(Ks, dtype=np.float32)
    As = np.asarray(As, dtype=np.float32)
    Ps = np.asarray(Ps, dtype=np.float32)

    mm_np = mybir.dt.np(MM_DT)
    ps_np = mybir.dt.np(PS_DT)

    nrm = np.sqrt(np.sum(Ks * Ks, axis=-1, keepdims=True))
    nK = Ks / np.maximum(nrm, EPS)
    w1T = np.ascontiguousarray((As * nK).transpose(2, 0, 1)).astype(mm_np, copy=False)
    w2T = np.zeros((D, E, KP), dtype=mybir.dt.np(W2_DT))
    w2T[:, :, :K] = (As * As).transpose(2, 0, 1).astype(w2T.dtype)

    ps_slices = [
        np.ascontiguousarray(
            Ps[:, :, si * LH : (si + 1) * LH, :].reshape(E, K, NDH)
        ).astype(ps_np, copy=False)
        for si in range(SSPLIT)
    ]
    xT = np.ascontiguousarray(x.T).astype(mm_np, copy=False)  # [D, B]

    in_maps = []
    for c in range(NCORES):
        si, q = divmod(c, QSPLIT)
        in_maps.append(
            {
                "xT": np.ascontiguousarray(xT[:, q * BC : (q + 1) * BC]),
                "w1T": w1T,
                "w2T": w2T,
                "ps": ps_slices[si],
            }
        )
    return in_maps


def _run(x, Ks, As, Ps, trace=False, **spmd_kwargs):
    nc = _get_nc()
    in_maps = _prep_inputs(x, Ks, As, Ps)
    res = run_bass_kernel_spmd(nc, in_maps, list(range(NCORES)), trace=trace, **spmd_kwargs)
    out = np.empty((2, E, B, L // 2, D), dtype=np.float32)
    for c in range(NCORES):
        si, q = divmod(c, QSPLIT)
        s, lp = divmod(si * LH, L // 2)
        out[s, :, q * BC : (q + 1) * BC, lp : lp + LH] = res.results[c]["out"].astype(
            np.float32
        )
    return out, res


def kernel(x, Ks, As, Ps):
    out, _ = _run(x, Ks, As, Ps, trace=False)
    return out
